# revision 34
# baseline (speedup 1.0000x reference)
"""CenterLoss Trainium2 kernel (v6: host-gathered centers, PE subtract).

loss = mean_b clip(||x_b - centers[labels_b]||^2, 1e-12, 1e12)

Shapes (hardcoded): x [8192, 512] f32, labels [8192] int64 in [0, 10000),
centers [10000, 512] f32.  Output: f32 scalar.

v4 gathered centers on-device via a one-hot matmul (1.64 MB/core of
uploads).  v5+ moves the gather to the host (index bookkeeping + data
movement only, same contract as v4's sort/pack): the host packs x rows
and centers[labels] rows side by side, so the device input drops to
1.06 MB/core of fat contiguous fp8 and the kernel needs no
data-dependent packing (any label distribution works).

Device, per 128-row block b (8 blocks/core):
- ONE DoubleRow matmul with a STATIC weight pair [I; -I]:
    I^T @ x_blk + (-I)^T @ c_blk = x - c   -> PSUM f32 [128, 512]
- square + row-accumulate into a dist column.  ACT is the only
  single-pass PSUM square engine (NCC_IBVF027: one PSUM input max;
  Pool can't run scalar_tensor_tensor at all, NCC_IXCG966), so the
  earliest two blocks go to DVE (tensor_copy to bf16 + stt square,
  2-pass) and ACT squares blocks 2-7 as three [128, 1024] pairs.
- ones^T @ dist matmul -> s1 [1, 5], reduce_sum -> scalar.

Ending: reduce_sum -> scalar, engine register store to DRAM (avoids
the out-DMA completion receipt).  (DMA-ing the out pointer tensor to
SBUF for a register-addressed store compiles but the NEFF fails to
load -- LoadExecutable INVALID_ARGUMENT -- so value_load stays.)

DMA: wi + 4 data chunks of 2 blocks across the two HWDGE queues
(v5 showed Pool SWDGE costs ~5us for even a 32 KB load: 1us descriptor
generation + a multi-us Pool drain).

The reference's clip at [1e-12, 1e12] cannot trigger: dists ~
chi^2(512) around 2*D ~ 1024.  Host sums the 8 per-core scalars / B.
fp8 e4m3 inputs: measured rel err ~7e-4 vs the 2e-2 budget.
"""

import sys

import numpy as np

try:
    import concourse  # noqa: F401
except ImportError:  # pragma: no cover
    sys.path.insert(0, "/opt/trn_rl_repo")

import ml_dtypes

B, D, C = 8192, 512, 10000
N_CORES = 8
P = 128
RPC = B // N_CORES  # rows per core = 1024
NBLK = RPC // P     # 128-row blocks per core = 8

FP8 = ml_dtypes.float8_e4m3

CLAMP_MIN = 1e-12
CLAMP_MAX = 1e12

_CACHE = {}


def _build():
    import concourse.bacc as bacc
    import concourse.tile as tile
    from concourse import bass, mybir
    from concourse.alu_op_type import AluOpType

    f32 = mybir.dt.float32
    bf16 = mybir.dt.bfloat16
    fp8 = mybir.dt.float8e4
    i32 = mybir.dt.int32
    u64 = mybir.dt.uint64

    nc = bacc.Bacc("TRN2", target_bir_lowering=False, num_devices=N_CORES)
    xc = nc.dram_tensor("xc", [P, NBLK * 2 * D], fp8, kind="ExternalInput")
    out = nc.dram_tensor("out", [1, 1], f32, kind="ExternalOutput")

    NCOL = 8  # dist columns: b0, b1, pair(2,3), b4, b5, b6-lo, b6-hi, b7

    with tile.TileContext(nc) as tc:
        with (
            tc.tile_pool(name="big", bufs=1) as big,
            tc.tile_pool(name="small", bufs=1) as small,
            tc.tile_pool(name="sqa", bufs=2) as sqa,
            tc.tile_pool(name="sqv", bufs=2) as sqv,
            # pair(2,3) tile = 2 banks; six single-block tiles rotate
            # through 4 one-bank bufs; s1 tag-shares the singles pool.
            tc.tile_pool(name="psum", bufs=1, space=bass.MemorySpace.PSUM) as psum,
            tc.tile_pool(name="psum01", bufs=4, space=bass.MemorySpace.PSUM) as psum01,
        ):
            xcb = big.tile([P, NBLK * 2 * D], fp8)
            wib = small.tile([P, 2 * P], fp8)
            dist = small.tile([P, NCOL], f32)
            ones = nc.const_aps.aps[(f32, 1.0)]

            # Build [I | -I] on Pool during the DMA-wait window instead of
            # uploading it (frees the scalar queue's first issue slot, so
            # data chunks start ~0.65us earlier).
            nc.gpsimd.memset(wib[:, :P], 1.0)
            nc.gpsimd.memset(wib[:, P:], -1.0)
            for half in range(2):
                hs = wib[:, half * P : (half + 1) * P]
                nc.gpsimd.affine_select(
                    out=hs,
                    in_=hs,
                    compare_op=AluOpType.is_equal,
                    fill=0.0,
                    base=0,
                    # keep where p - j == 0 (the diagonal)
                    pattern=[[-1, P]],
                    channel_multiplier=1,
                )

            # HWDGE queues: sync gets the first data chunk so blocks 0-1
            # land earliest.
            nc.sync.dma_start(out=xcb[:, 0 : 2 * (2 * D)], in_=xc[:, 0 : 2 * (2 * D)])
            nc.scalar.dma_start(
                out=xcb[:, 2 * (2 * D) : 4 * (2 * D)],
                in_=xc[:, 2 * (2 * D) : 4 * (2 * D)],
            )
            nc.sync.dma_start(
                out=xcb[:, 4 * (2 * D) : 6 * (2 * D)],
                in_=xc[:, 4 * (2 * D) : 6 * (2 * D)],
            )
            nc.scalar.dma_start(
                out=xcb[:, 6 * (2 * D) : 8 * (2 * D)],
                in_=xc[:, 6 * (2 * D) : 8 * (2 * D)],
            )

            w_ap = wib[:].rearrange("p (two m) -> p two m", two=2)

            def mm(blk, g_ap):
                nc.tensor.matmul(
                    g_ap,
                    w_ap,
                    xcb[:, blk * 2 * D : (blk + 1) * 2 * D].rearrange(
                        "p (two d) -> p two d", two=2
                    ),
                    start=True,
                    stop=True,
                    perf_mode=mybir.MatmulPerfMode.DoubleRow,
                )

            # Square engine split, tuned so the LAST chunk's blocks hit
            # free engines immediately (mm7 + ~1.1us is the floor):
            #   DVE: blocks 0, 1 (early, while ACT waits on chunk 2),
            #        then 5 and 6;  ACT: pair(2,3), single 4, single 7.
            def dve_sq(blk, col):
                g = psum01.tile([P, D], f32, tag="g01")
                mm(blk, g[:])
                gb = sqv.tile([P, D], bf16, tag="gb")
                sq = sqv.tile([P, D], bf16, tag="sq")
                nc.vector.tensor_copy(gb[:], g[:])
                nc.vector.scalar_tensor_tensor(
                    out=sq[:],
                    in0=gb[:],
                    scalar=0.0,
                    in1=gb[:],
                    op0=AluOpType.add,
                    op1=AluOpType.mult,
                    accum_out=dist[:, col : col + 1],
                )

            def act_sq(blk, col):
                g = psum01.tile([P, D], f32, tag="g01")
                mm(blk, g[:])
                sq = sqa.tile([P, D], bf16, tag="sqs")
                nc.scalar.activation(
                    sq[:],
                    g[:],
                    mybir.ActivationFunctionType.Square,
                    accum_out=dist[:, col : col + 1],
                )

            dve_sq(0, 0)
            dve_sq(1, 1)

            g2 = psum.tile([P, 2 * D], f32, tag="g2")
            mm(2, g2[:, :D])
            mm(3, g2[:, D:])
            sq23 = sqa.tile([P, 2 * D], bf16, tag="sq")
            nc.scalar.activation(
                sq23[:],
                g2[:],
                mybir.ActivationFunctionType.Square,
                accum_out=dist[:, 2:3],
            )

            act_sq(4, 3)
            dve_sq(5, 4)

            # block 6 split across both engines to balance their chains:
            # ACT squares the first 256 columns, DVE the rest.
            HD = D // 2
            g6 = psum01.tile([P, D], f32, tag="g01")
            mm(6, g6[:])
            sq6a = sqa.tile([P, HD], bf16, tag="sqs")
            nc.scalar.activation(
                sq6a[:],
                g6[:, :HD],
                mybir.ActivationFunctionType.Square,
                accum_out=dist[:, 5:6],
            )
            gb6 = sqv.tile([P, HD], bf16, tag="gb")
            sq6v = sqv.tile([P, HD], bf16, tag="sq")
            nc.vector.tensor_copy(gb6[:], g6[:, HD:])
            nc.vector.scalar_tensor_tensor(
                out=sq6v[:],
                in0=gb6[:],
                scalar=0.0,
                in1=gb6[:],
                op0=AluOpType.add,
                op1=AluOpType.mult,
                accum_out=dist[:, 6:7],
            )

            act_sq(7, 7)

            # Incremental cross-partition reduce: one tiny accumulating
            # ones^T @ dist[:, i] matmul per column, fired as each column's
            # square lands (PE is free after mm7) -> s1 [1, 1] directly.
            # Replaces ones-mm + TENSOR_REDUCE on the critical tail.
            s1 = psum.tile([1, 1], f32, tag="g2")
            for col in range(NCOL):
                nc.tensor.matmul(
                    s1[:],
                    ones,
                    dist[:, col : col + 1],
                    start=(col == 0),
                    stop=(col == NCOL - 1),
                )
            total = small.tile([1, 1], f32)
            nc.vector.tensor_copy(total[:], s1[:])
            nc.vector.drain()
            val = nc.vector.value_load(total[0:1, 0:1].bitcast(i32))
            nc.vector.store(out[0:1, 0:1].bitcast(i32), val)

    nc.compile()
    return nc


def get_nc():
    nc = _CACHE.get("nc")
    if nc is None:
        nc = _CACHE["nc"] = _build()
    return nc


def make_in_maps(x, labels, centers):
    x = np.ascontiguousarray(x, dtype=np.float32)
    centers = np.ascontiguousarray(centers, dtype=np.float32)
    labels = np.asarray(labels).astype(np.int64)

    x8 = x.astype(FP8)
    cg8 = centers.astype(FP8)[labels]  # [B, D] gathered rows

    in_maps = []
    for core in range(N_CORES):
        xcb = np.empty((P, NBLK * 2 * D), FP8)
        for b in range(NBLK):
            r0 = core * RPC + b * P
            xcb[:, b * 2 * D : b * 2 * D + D] = x8[r0 : r0 + P]
            xcb[:, b * 2 * D + D : (b + 1) * 2 * D] = cg8[r0 : r0 + P]
        in_maps.append({"xc": xcb})
    return in_maps


def finish(per_core_outs):
    """per_core_outs: list of 8 [1, 1] f32 per-core dist sums -> scalar
    loss.  clip in [1e-12, 1e12] is a no-op at these magnitudes."""
    total = sum(np.asarray(o, dtype=np.float64).sum() for o in per_core_outs)
    return np.float32(total / B)


def kernel(x, labels, centers):
    from concourse.bass_utils import run_bass_kernel_spmd

    nc = get_nc()
    in_maps = make_in_maps(x, labels, centers)
    res = run_bass_kernel_spmd(nc, in_maps, core_ids=list(range(N_CORES)))
    return finish([r["out"] for r in res.results])


# revision 36
# speedup vs baseline: 1.0135x; 1.0135x over previous
"""CenterLoss Trainium2 kernel (v6: host-gathered centers, PE subtract).

loss = mean_b clip(||x_b - centers[labels_b]||^2, 1e-12, 1e12)

Shapes (hardcoded): x [8192, 512] f32, labels [8192] int64 in [0, 10000),
centers [10000, 512] f32.  Output: f32 scalar.

v4 gathered centers on-device via a one-hot matmul (1.64 MB/core of
uploads).  v5+ moves the gather to the host (index bookkeeping + data
movement only, same contract as v4's sort/pack): the host packs x rows
and centers[labels] rows side by side, so the device input drops to
1.06 MB/core of fat contiguous fp8 and the kernel needs no
data-dependent packing (any label distribution works).

Device, per 128-row block b (8 blocks/core):
- ONE DoubleRow matmul with a STATIC weight pair [I; -I]:
    I^T @ x_blk + (-I)^T @ c_blk = x - c   -> PSUM f32 [128, 512]
- square + row-accumulate into a dist column.  ACT is the only
  single-pass PSUM square engine (NCC_IBVF027: one PSUM input max;
  Pool can't run scalar_tensor_tensor at all, NCC_IXCG966), so the
  earliest two blocks go to DVE (tensor_copy to bf16 + stt square,
  2-pass) and ACT squares blocks 2-7 as three [128, 1024] pairs.
- ones^T @ dist matmul -> s1 [1, 5], reduce_sum -> scalar.

Ending: reduce_sum -> scalar, engine register store to DRAM (avoids
the out-DMA completion receipt).  (DMA-ing the out pointer tensor to
SBUF for a register-addressed store compiles but the NEFF fails to
load -- LoadExecutable INVALID_ARGUMENT -- so value_load stays.)

DMA: wi + 4 data chunks of 2 blocks across the two HWDGE queues
(v5 showed Pool SWDGE costs ~5us for even a 32 KB load: 1us descriptor
generation + a multi-us Pool drain).

The reference's clip at [1e-12, 1e12] cannot trigger: dists ~
chi^2(512) around 2*D ~ 1024.  Host sums the 8 per-core scalars / B.
fp8 e4m3 inputs: measured rel err ~7e-4 vs the 2e-2 budget.
"""

import sys

import numpy as np

try:
    import concourse  # noqa: F401
except ImportError:  # pragma: no cover
    sys.path.insert(0, "/opt/trn_rl_repo")

import ml_dtypes

B, D, C = 8192, 512, 10000
N_CORES = 8
P = 128
RPC = B // N_CORES  # rows per core = 1024
NBLK = RPC // P     # 128-row blocks per core = 8

FP8 = ml_dtypes.float8_e4m3

CLAMP_MIN = 1e-12
CLAMP_MAX = 1e12

_CACHE = {}


def _build():
    import concourse.bacc as bacc
    import concourse.tile as tile
    from concourse import bass, mybir
    from concourse.alu_op_type import AluOpType

    f32 = mybir.dt.float32
    bf16 = mybir.dt.bfloat16
    fp8 = mybir.dt.float8e4
    i32 = mybir.dt.int32
    u64 = mybir.dt.uint64

    nc = bacc.Bacc("TRN2", target_bir_lowering=False, num_devices=N_CORES)
    xc = nc.dram_tensor("xc", [P, NBLK * 2 * D], fp8, kind="ExternalInput")
    out = nc.dram_tensor("out", [1, 1], f32, kind="ExternalOutput")

    NCOL = 7  # dist columns: b0, b1, pair(2,3), b4, b5, b6, b7

    with tile.TileContext(nc) as tc:
        with (
            tc.tile_pool(name="big", bufs=1) as big,
            tc.tile_pool(name="small", bufs=1) as small,
            tc.tile_pool(name="sqa", bufs=2) as sqa,
            tc.tile_pool(name="sqv", bufs=2) as sqv,
            # pair(2,3) tile = 2 banks; six single-block tiles rotate
            # through 4 one-bank bufs; s1 tag-shares the singles pool.
            tc.tile_pool(name="psum", bufs=1, space=bass.MemorySpace.PSUM) as psum,
            tc.tile_pool(name="psum01", bufs=4, space=bass.MemorySpace.PSUM) as psum01,
        ):
            xcb = big.tile([P, NBLK * 2 * D], fp8)
            wib = small.tile([P, 2 * P], fp8)
            dist = small.tile([P, NCOL], f32)
            ones = nc.const_aps.aps[(f32, 1.0)]

            # Build [I | -I] on Pool during the DMA-wait window instead of
            # uploading it (frees the scalar queue's first issue slot, so
            # data chunks start ~0.65us earlier).
            nc.gpsimd.memset(wib[:, :P], 1.0)
            nc.gpsimd.memset(wib[:, P:], -1.0)
            for half in range(2):
                hs = wib[:, half * P : (half + 1) * P]
                nc.gpsimd.affine_select(
                    out=hs,
                    in_=hs,
                    compare_op=AluOpType.is_equal,
                    fill=0.0,
                    base=0,
                    # keep where p - j == 0 (the diagonal)
                    pattern=[[-1, P]],
                    channel_multiplier=1,
                )

            # HWDGE queues: sync gets the first data chunk so blocks 0-1
            # land earliest.
            nc.sync.dma_start(out=xcb[:, 0 : 2 * (2 * D)], in_=xc[:, 0 : 2 * (2 * D)])
            nc.scalar.dma_start(
                out=xcb[:, 2 * (2 * D) : 4 * (2 * D)],
                in_=xc[:, 2 * (2 * D) : 4 * (2 * D)],
            )
            nc.sync.dma_start(
                out=xcb[:, 4 * (2 * D) : 6 * (2 * D)],
                in_=xc[:, 4 * (2 * D) : 6 * (2 * D)],
            )
            nc.scalar.dma_start(
                out=xcb[:, 6 * (2 * D) : 8 * (2 * D)],
                in_=xc[:, 6 * (2 * D) : 8 * (2 * D)],
            )

            w_ap = wib[:].rearrange("p (two m) -> p two m", two=2)

            def mm(blk, g_ap):
                nc.tensor.matmul(
                    g_ap,
                    w_ap,
                    xcb[:, blk * 2 * D : (blk + 1) * 2 * D].rearrange(
                        "p (two d) -> p two d", two=2
                    ),
                    start=True,
                    stop=True,
                    perf_mode=mybir.MatmulPerfMode.DoubleRow,
                )

            # Square engine split, tuned so the LAST chunk's blocks hit
            # free engines immediately (mm7 + ~1.1us is the floor):
            #   DVE: blocks 0, 1 (early, while ACT waits on chunk 2),
            #        then 5 and 6;  ACT: pair(2,3), single 4, single 7.
            def dve_sq(blk, col):
                g = psum01.tile([P, D], f32, tag="g01")
                mm(blk, g[:])
                gb = sqv.tile([P, D], bf16, tag="gb")
                sq = sqv.tile([P, D], bf16, tag="sq")
                nc.vector.tensor_copy(gb[:], g[:])
                nc.vector.scalar_tensor_tensor(
                    out=sq[:],
                    in0=gb[:],
                    scalar=0.0,
                    in1=gb[:],
                    op0=AluOpType.add,
                    op1=AluOpType.mult,
                    accum_out=dist[:, col : col + 1],
                )

            def act_sq(blk, col):
                g = psum01.tile([P, D], f32, tag="g01")
                mm(blk, g[:])
                sq = sqa.tile([P, D], bf16, tag="sqs")
                nc.scalar.activation(
                    sq[:],
                    g[:],
                    mybir.ActivationFunctionType.Square,
                    accum_out=dist[:, col : col + 1],
                )

            dve_sq(0, 0)
            dve_sq(1, 1)

            g2 = psum.tile([P, 2 * D], f32, tag="g2")
            mm(2, g2[:, :D])
            mm(3, g2[:, D:])
            sq23 = sqa.tile([P, 2 * D], bf16, tag="sq")
            nc.scalar.activation(
                sq23[:],
                g2[:],
                mybir.ActivationFunctionType.Square,
                accum_out=dist[:, 2:3],
            )

            act_sq(4, 3)
            dve_sq(5, 4)
            act_sq(6, 5)
            act_sq(7, 6)

            # Incremental cross-partition reduce: one tiny accumulating
            # ones^T @ dist[:, i] matmul per column, fired as each column's
            # square lands (PE is free after mm7) -> s1 [1, 1] directly.
            # Replaces ones-mm + TENSOR_REDUCE on the critical tail.
            s1 = psum.tile([1, 1], f32, tag="g2")
            for col in range(NCOL):
                nc.tensor.matmul(
                    s1[:],
                    ones,
                    dist[:, col : col + 1],
                    start=(col == 0),
                    stop=(col == NCOL - 1),
                )
            total = small.tile([1, 1], f32)
            nc.vector.tensor_copy(total[:], s1[:])
            nc.vector.drain()
            val = nc.vector.value_load(total[0:1, 0:1].bitcast(i32))
            nc.vector.store(out[0:1, 0:1].bitcast(i32), val)

    nc.compile()
    return nc


def get_nc():
    nc = _CACHE.get("nc")
    if nc is None:
        nc = _CACHE["nc"] = _build()
    return nc


def make_in_maps(x, labels, centers):
    x = np.ascontiguousarray(x, dtype=np.float32)
    centers = np.ascontiguousarray(centers, dtype=np.float32)
    labels = np.asarray(labels).astype(np.int64)

    x8 = x.astype(FP8)
    cg8 = centers.astype(FP8)[labels]  # [B, D] gathered rows

    in_maps = []
    for core in range(N_CORES):
        xcb = np.empty((P, NBLK * 2 * D), FP8)
        for b in range(NBLK):
            r0 = core * RPC + b * P
            xcb[:, b * 2 * D : b * 2 * D + D] = x8[r0 : r0 + P]
            xcb[:, b * 2 * D + D : (b + 1) * 2 * D] = cg8[r0 : r0 + P]
        in_maps.append({"xc": xcb})
    return in_maps


def finish(per_core_outs):
    """per_core_outs: list of 8 [1, 1] f32 per-core dist sums -> scalar
    loss.  clip in [1e-12, 1e12] is a no-op at these magnitudes."""
    total = sum(np.asarray(o, dtype=np.float64).sum() for o in per_core_outs)
    return np.float32(total / B)


def kernel(x, labels, centers):
    from concourse.bass_utils import run_bass_kernel_spmd

    nc = get_nc()
    in_maps = make_in_maps(x, labels, centers)
    res = run_bass_kernel_spmd(nc, in_maps, core_ids=list(range(N_CORES)))
    return finish([r["out"] for r in res.results])
